# revision 35
# baseline (speedup 1.0000x reference)
"""CapsuleLayer (dynamic routing) Trainium2 kernel, SPMD over 8 NeuronCores.

Sharding: input-capsule axis (IN_CAPS=512 -> 64 per core). W and u_hat are
i-sharded; the bij,bijd->bjd contraction is completed with a bf16 AllReduce
of folded s-partials (64x16x32, 65 KB/rank) once per routing iteration. The
CCE in the SDMA datapath does the rank-sum, so readback is two contiguous
DMAs straight into both partition halves.

Per-core layout (i_local = i2*32 + i1, i2 in {0,1}):
  u_hat SBUF [p=(i2*64+b), (d, i1, j)] bf16 -- both broadcast multiplies
        (c over outermost d, outputs over middle i1) keep the DVE bf16 fast
        mode, and both contractions are contiguous in-place tree-adds
  b/c logits [p, (i1, j)]; s/outputs live on all 128 partitions (both halves
        hold the full batch) so squash and the b-update need no
        partition-shift inside the loop.

Phase 1 (per i): u_hat_i[b, dj] = xT_i.T @ W_i on the PE (K=128, M=64,
N=512), all in bf16.

The partition-half fold of the s-partial rides the bounce-buffer DMAs: the
upper half DMAs to DRAM while the lower half's add runs, then the lower
half lands with a CCE-accumulate DMA. A tiny warmup AllReduce issued at
kernel start absorbs the collective stack's ~60us background init plus the
first-collective premium off the critical path (measured: without it the
first real collective pays ~22us instead of ~9).
"""

import numpy as np

N_CORES = 8
B = 64
IN_CAPS = 512
IN_DIM = 128
N_CAPS = 16
OUT_DIM = 32
I_LOC = IN_CAPS // N_CORES          # 64 input capsules per core
I1 = 32                             # i_local = i2*32 + i1
JD = N_CAPS * OUT_DIM               # 512
EPS = 1e-7
GRP = 4                             # i's per W-DMA/PSUM group
NGRP = I_LOC // GRP                 # 16

# Toggled by test.py for profiling runs.
TRACE = False
TRACE_DIR = None

_cache = {}


def _emit(tc, xT, wT, out, num_routing):
    from contextlib import ExitStack

    from concourse import mybir

    nc = tc.nc
    f32 = mybir.dt.float32
    bf16 = mybir.dt.bfloat16
    ctx = ExitStack()
    singles = ctx.enter_context(tc.tile_pool(name="singles", bufs=1))
    wpool = ctx.enter_context(tc.tile_pool(name="wpool", bufs=4))
    pspool = ctx.enter_context(tc.tile_pool(name="pspool", bufs=2, space="PSUM"))
    small = ctx.enter_context(tc.tile_pool(name="small", bufs=2))
    gath = ctx.enter_context(tc.tile_pool(name="gath", bufs=1))
    dram = ctx.enter_context(tc.tile_pool(name="dram", bufs=2, space="DRAM"))

    # One tiny warmup collective: the collective stack finishes its background
    # init ~68us into the kernel and charges a first-collective premium; a
    # tiny collective absorbs both off the critical path (measured: without
    # it the r0 collective pays ~22us instead of ~7). AllGather warms the
    # same ncfw/mesh path but completes ~3us faster than an AllReduce.
    warm_in = dram.tile([1, 32], f32)
    warm_out = dram.tile([8, 32], f32)
    nc.gpsimd.collective_compute(
        "AllGather",
        mybir.AluOpType.bypass,
        replica_groups=[list(range(N_CORES))],
        ins=[warm_in.opt()],
        outs=[warm_out.opt()],
    )

    # ---- phase 1: u_hat = einsum over k, per local capsule i ----
    xsb = singles.tile([IN_DIM, I_LOC, B], bf16)         # [k, i, b]
    u_hat = singles.tile([128, OUT_DIM, I1, N_CAPS], bf16)  # [(i2,b), d, i1, j]

    XCH = I_LOC // 4
    for g in range(NGRP):
        i2 = (g * GRP) // I1
        i1g = (g * GRP) % I1
        # interleave the x chunks with the first W groups so the first
        # matmul's operands land as early as possible
        if g < 4:
            q = g
            nc.sync.dma_start(
                xsb[:, q * XCH:(q + 1) * XCH, :],
                xT[:, q * XCH:(q + 1) * XCH, :],
            )
        wtile = wpool.tile([IN_DIM, GRP, OUT_DIM, N_CAPS], bf16)
        nc.sync.dma_start(wtile[:], wT[g])
        ps = pspool.tile([128, GRP, OUT_DIM, N_CAPS], f32)
        for t in range(GRP):
            i = g * GRP + t
            nc.tensor.matmul(
                ps[i2 * B:(i2 + 1) * B, t], xsb[:, i, :], wtile[:, t],
                start=True, stop=True,
            )
        # copy+cast PSUM f32 -> SBUF bf16; dst viewed (i1, d, j) to match src
        dst = u_hat[i2 * B:(i2 + 1) * B, :, i1g:i1g + GRP, :].transpose(
            [0, 2, 1, 3]
        )
        src = ps[i2 * B:(i2 + 1) * B]
        if g % 2 == 0:
            nc.vector.tensor_copy(out=dst, in_=src)
        else:
            nc.scalar.copy(out=dst, in_=src)

    # ---- phase 2: routing ----
    tmp = singles.tile([128, OUT_DIM, I1, N_CAPS], bf16)
    b_log = singles.tile([128, I1, N_CAPS], f32)
    out2 = singles.tile([128, OUT_DIM, N_CAPS], bf16)    # squash(s) as (d, j)
    eps_t = singles.tile([128, 1], f32)
    nc.vector.memset(b_log[:], 0.0)
    nc.vector.memset(eps_t[:], EPS)

    R = num_routing
    for r in range(R):
        if r == 0:
            # b == 0 -> c uniform: s = (1/16) * sum_i u_hat (scale after AG)
            nc.vector.tensor_add(
                tmp[:, :, :I1 // 2], u_hat[:, :, :I1 // 2],
                u_hat[:, :, I1 // 2:],
            )
        else:
            # |b| stays < ~20 for this distribution: exp is fp32-safe
            # without the max-subtraction
            cexp = small.tile([128, I1, N_CAPS], f32)
            nc.scalar.activation(
                out=cexp[:], in_=b_log[:],
                func=mybir.ActivationFunctionType.Exp,
            )
            csum = small.tile([128, I1], f32)
            nc.vector.reduce_sum(
                out=csum[:], in_=cexp[:], axis=mybir.AxisListType.X
            )
            nc.vector.reciprocal(out=csum[:], in_=csum[:])
            c_t = small.tile([128, I1, N_CAPS], bf16)
            nc.vector.tensor_mul(
                c_t[:], cexp[:],
                csum.unsqueeze(2).broadcast_to([128, I1, N_CAPS]),
            )
            # s-mul: broadcast c over outermost d keeps bf16 2x mode
            nc.vector.tensor_mul(
                tmp[:], u_hat[:],
                c_t.unsqueeze(1).broadcast_to([128, OUT_DIM, I1, N_CAPS]),
            )
            nc.vector.tensor_add(
                tmp[:, :, :I1 // 2], tmp[:, :, :I1 // 2], tmp[:, :, I1 // 2:]
            )
        # contiguous in-place tree over i1 (middle dim)
        w = I1 // 2
        while w > 2:
            nc.vector.tensor_add(
                tmp[:, :, :w // 2], tmp[:, :, :w // 2], tmp[:, :, w // 2:w]
            )
            w //= 2
        # final tree level split per partition half: the upper half lands
        # first so its DMA to the bounce buffer overlaps the lower half's
        # add; the partition fold happens in the second DMA via the CCE
        # accumulate in the SDMA datapath
        s_half = small.tile([128, OUT_DIM, N_CAPS], bf16)
        nc.vector.tensor_add(
            s_half[B:2 * B], tmp[B:2 * B, :, 0, :], tmp[B:2 * B, :, 1, :]
        )
        cc_in = dram.tile([B, OUT_DIM, N_CAPS], bf16)
        cc_out = dram.tile(
            [B, OUT_DIM, N_CAPS], bf16, addr_space="Shared"
        )
        nc.sync.dma_start(cc_in[:], s_half[B:2 * B])
        nc.vector.tensor_add(
            s_half[0:B], tmp[0:B, :, 0, :], tmp[0:B, :, 1, :]
        )
        nc.gpsimd.dma_start(
            cc_in[:], s_half[0:B], accum_op=mybir.AluOpType.add
        )
        nc.gpsimd.collective_compute(
            "AllReduce",
            mybir.AluOpType.add,
            replica_groups=[list(range(N_CORES))],
            ins=[cc_in.opt()],
            outs=[cc_out.opt()],
        )
        # readback: both partition halves get the full sum so squash and the
        # next b-update run on all 128 partitions with no further shifts
        # (final iteration only needs the lower half)
        last = r == R - 1
        PH = B if last else 2 * B
        # readback on sync+gpsimd: the scalar queue may be busy with the
        # ACT_TABLE_LOAD for the upcoming sqrt and would delay its DMA
        s_sb = gath.tile([128, OUT_DIM, N_CAPS], bf16)
        nc.sync.dma_start(s_sb[0:B], cc_out[:])
        if not last:
            nc.gpsimd.dma_start(s_sb[B:2 * B], cc_out[:])
        # iteration 0's uniform c = 1/16 is folded into the squash math:
        # with s' = 16*s, ss = sum_d (s'/16)^2 and out = f(ss) * (s'/16)
        inv = 1.0 / N_CAPS if r == 0 else 1.0

        # squash: scale = ss/(1+ss)/sqrt(ss+eps), ss = sum_d s^2
        sq = small.tile([128, OUT_DIM, N_CAPS], f32)
        nc.vector.scalar_tensor_tensor(
            sq[0:PH], s_sb[0:PH], inv * inv, s_sb[0:PH],
            mybir.AluOpType.mult, mybir.AluOpType.mult,
        )
        ss = small.tile([128, N_CAPS], f32)
        nc.vector.reduce_sum(
            out=ss[0:PH], in_=sq[0:PH].transpose([0, 2, 1]),
            axis=mybir.AxisListType.X,
        )
        t1 = small.tile([128, N_CAPS], f32)
        nc.scalar.activation(
            out=t1[0:PH], in_=ss[0:PH],
            func=mybir.ActivationFunctionType.Sqrt,
            bias=eps_t[0:PH], scale=1.0,
        )
        t2 = small.tile([128, N_CAPS], f32)
        nc.vector.scalar_tensor_tensor(
            t2[0:PH], ss[0:PH], 1.0, t1[0:PH],
            mybir.AluOpType.add, mybir.AluOpType.mult,
        )   # (1+ss)*sqrt(ss+eps)
        nc.vector.reciprocal(out=t2[0:PH], in_=t2[0:PH])
        nc.vector.scalar_tensor_tensor(
            t1[0:PH], ss[0:PH], inv, t2[0:PH],
            mybir.AluOpType.mult, mybir.AluOpType.mult,
        )   # t1 = (ss*inv) / ((1+ss)*sqrt(ss+eps)) -- squash scale (*inv)
        if r == R - 1:
            out_t = small.tile([B, N_CAPS, OUT_DIM], f32)
            nc.vector.tensor_mul(
                out_t[:], s_sb[0:B].transpose([0, 2, 1]),
                t1[0:B].unsqueeze(2).broadcast_to([B, N_CAPS, OUT_DIM]),
            )
            nc.sync.dma_start(out[:], out_t[:])
        else:
            nc.vector.tensor_mul(
                out2[:], s_sb[:],
                t1.unsqueeze(1).broadcast_to([128, OUT_DIM, N_CAPS]),
            )
            # bu-mul: broadcast outputs over middle i1 keeps bf16 2x mode
            nc.vector.tensor_mul(
                tmp[:], u_hat[:],
                out2.unsqueeze(2).broadcast_to([128, OUT_DIM, I1, N_CAPS]),
            )
            w = OUT_DIM
            while w > 2:
                nc.vector.tensor_add(
                    tmp[:, :w // 2], tmp[:, :w // 2], tmp[:, w // 2:w]
                )
                w //= 2
            if r == 0:
                # b was zero: write the fresh logits directly
                nc.vector.tensor_add(b_log[:], tmp[:, 0], tmp[:, 1])
            else:
                bred = small.tile([128, I1, N_CAPS], f32)
                nc.vector.tensor_add(bred[:], tmp[:, 0], tmp[:, 1])
                nc.vector.tensor_add(b_log[:], b_log[:], bred[:])

    ctx.close()


def _build(num_routing):
    import concourse.bacc as bacc
    import concourse.tile as tile
    from concourse import mybir

    nc = bacc.Bacc(
        "TRN2", target_bir_lowering=False, debug=False, num_devices=N_CORES,
        dynamic_dma_scratch_size=512,
    )
    f32 = mybir.dt.float32
    bf16 = mybir.dt.bfloat16
    xT = nc.dram_tensor("xT", [IN_DIM, I_LOC, B], bf16, kind="ExternalInput")
    wT = nc.dram_tensor(
        "wT", [NGRP, IN_DIM, GRP, OUT_DIM, N_CAPS], bf16, kind="ExternalInput"
    )
    out = nc.dram_tensor(
        "out", [B, N_CAPS, OUT_DIM], f32, kind="ExternalOutput"
    )
    with tile.TileContext(nc) as tc:
        _emit(tc, xT, wT, out, num_routing)
    nc.compile()
    return nc


def kernel(inputs, W, num_routing):
    import ml_dtypes

    from concourse.bass_utils import run_bass_kernel_spmd

    R = int(num_routing)
    assert R >= 1
    if R not in _cache:
        _cache[R] = _build(R)
    nc = _cache[R]

    bf = ml_dtypes.bfloat16
    inputs = np.ascontiguousarray(np.asarray(inputs, dtype=np.float32))
    W = np.asarray(W, dtype=np.float32)

    in_maps = []
    for c in range(N_CORES):
        lo, hi = c * I_LOC, (c + 1) * I_LOC
        xT_c = np.ascontiguousarray(
            inputs[:, lo:hi, :].transpose(2, 1, 0).astype(bf)
        )
        # [i,j,k,d] -> group-blocked [g, k, t, d, j] so each group DMA is one
        # contiguous block and PSUM columns come out in (d, j) order
        wT_c = np.ascontiguousarray(
            W[lo:hi]
            .reshape(NGRP, GRP, N_CAPS, IN_DIM, OUT_DIM)
            .transpose(0, 3, 1, 4, 2)
            .astype(bf)
        )
        in_maps.append({"xT": xT_c, "wT": wT_c})

    kwargs = {}
    if TRACE:
        kwargs["trace"] = True
        if TRACE_DIR:
            kwargs["tmpdir"] = TRACE_DIR
    res = None
    for attempt in range(3):
        try:
            res = run_bass_kernel_spmd(
                nc, in_maps, core_ids=list(range(N_CORES)), **kwargs
            )
            break
        except Exception:
            if attempt == 2:
                raise
            import time
            time.sleep(5)
    if TRACE:
        kernel.last_exec_time_ns = res.exec_time_ns
        kernel.last_results = res
    return np.asarray(res.results[0]["out"], dtype=np.float32)


# revision 36
# speedup vs baseline: 1.0152x; 1.0152x over previous
"""CapsuleLayer (dynamic routing) Trainium2 kernel, SPMD over 8 NeuronCores.

Sharding: input-capsule axis (IN_CAPS=512 -> 64 per core). W and u_hat are
i-sharded; the bij,bijd->bjd contraction is completed with a bf16 AllReduce
of folded s-partials (64x16x32, 65 KB/rank) once per routing iteration. The
CCE in the SDMA datapath does the rank-sum, so readback is two contiguous
DMAs straight into both partition halves.

Per-core layout (i_local = i2*32 + i1, i2 in {0,1}):
  u_hat SBUF [p=(i2*64+b), (d, i1, j)] bf16 -- both broadcast multiplies
        (c over outermost d, outputs over middle i1) keep the DVE bf16 fast
        mode, and both contractions are contiguous in-place tree-adds
  b/c logits [p, (i1, j)]; s/outputs live on all 128 partitions (both halves
        hold the full batch) so squash and the b-update need no
        partition-shift inside the loop.

Phase 1 (per i): u_hat_i[b, dj] = xT_i.T @ W_i on the PE (K=128, M=64,
N=512), all in bf16.

The partition-half fold of the s-partial rides the bounce-buffer DMAs: the
upper half DMAs to DRAM while the lower half's add runs, then the lower
half lands with a CCE-accumulate DMA. A tiny warmup AllReduce issued at
kernel start absorbs the collective stack's ~60us background init plus the
first-collective premium off the critical path (measured: without it the
first real collective pays ~22us instead of ~9).
"""

import numpy as np

N_CORES = 8
B = 64
IN_CAPS = 512
IN_DIM = 128
N_CAPS = 16
OUT_DIM = 32
I_LOC = IN_CAPS // N_CORES          # 64 input capsules per core
I1 = 32                             # i_local = i2*32 + i1
JD = N_CAPS * OUT_DIM               # 512
EPS = 1e-7
GRP = 4                             # i's per W-DMA/PSUM group
NGRP = I_LOC // GRP                 # 16

# Toggled by test.py for profiling runs.
TRACE = False
TRACE_DIR = None

_cache = {}


def _emit(tc, xT, wT, out, num_routing):
    from contextlib import ExitStack

    from concourse import mybir

    nc = tc.nc
    f32 = mybir.dt.float32
    bf16 = mybir.dt.bfloat16
    ctx = ExitStack()
    singles = ctx.enter_context(tc.tile_pool(name="singles", bufs=1))
    wpool = ctx.enter_context(tc.tile_pool(name="wpool", bufs=4))
    pspool = ctx.enter_context(tc.tile_pool(name="pspool", bufs=2, space="PSUM"))
    small = ctx.enter_context(tc.tile_pool(name="small", bufs=2))
    gath = ctx.enter_context(tc.tile_pool(name="gath", bufs=1))
    dram = ctx.enter_context(tc.tile_pool(name="dram", bufs=2, space="DRAM"))

    # One tiny warmup collective: the collective stack finishes its background
    # init ~68us into the kernel and charges a first-collective premium; a
    # tiny collective absorbs both off the critical path (measured: without
    # it the r0 collective pays ~22us instead of ~7). AllGather warms the
    # same ncfw/mesh path but completes ~3us faster than an AllReduce.
    warm_in = dram.tile([1, 32], f32)
    warm_out = dram.tile([8, 32], f32)
    nc.gpsimd.collective_compute(
        "AllGather",
        mybir.AluOpType.bypass,
        replica_groups=[list(range(N_CORES))],
        ins=[warm_in.opt()],
        outs=[warm_out.opt()],
    )

    # ---- phase 1: u_hat = einsum over k, per local capsule i ----
    xsb = singles.tile([IN_DIM, I_LOC, B], bf16)         # [k, i, b]
    u_hat = singles.tile([128, OUT_DIM, I1, N_CAPS], bf16)  # [(i2,b), d, i1, j]

    XCH = I_LOC // 4
    for g in range(NGRP):
        i2 = (g * GRP) // I1
        i1g = (g * GRP) % I1
        # interleave the x chunks with the first W groups so the first
        # matmul's operands land as early as possible
        if g < 4:
            q = g
            nc.sync.dma_start(
                xsb[:, q * XCH:(q + 1) * XCH, :],
                xT[:, q * XCH:(q + 1) * XCH, :],
            )
        wtile = wpool.tile([IN_DIM, GRP, OUT_DIM, N_CAPS], bf16)
        nc.sync.dma_start(wtile[:], wT[g])
        ps = pspool.tile([128, GRP, OUT_DIM, N_CAPS], f32)
        for t in range(GRP):
            i = g * GRP + t
            nc.tensor.matmul(
                ps[i2 * B:(i2 + 1) * B, t], xsb[:, i, :], wtile[:, t],
                start=True, stop=True,
            )
        # copy+cast PSUM f32 -> SBUF bf16; dst viewed (i1, d, j) to match src
        dst = u_hat[i2 * B:(i2 + 1) * B, :, i1g:i1g + GRP, :].transpose(
            [0, 2, 1, 3]
        )
        src = ps[i2 * B:(i2 + 1) * B]
        if g % 2 == 0:
            nc.vector.tensor_copy(out=dst, in_=src)
        else:
            nc.scalar.copy(out=dst, in_=src)

    # ---- phase 2: routing ----
    tmp = singles.tile([128, OUT_DIM, I1, N_CAPS], bf16)
    b_log = singles.tile([128, I1, N_CAPS], f32)
    out2 = singles.tile([128, OUT_DIM, N_CAPS], bf16)    # squash(s) as (d, j)
    eps_t = singles.tile([128, 1], f32)
    nc.vector.memset(b_log[:], 0.0)
    nc.vector.memset(eps_t[:], EPS)

    R = num_routing
    for r in range(R):
        if r == 0:
            # b == 0 -> c uniform: s = (1/16) * sum_i u_hat (scale after AG)
            nc.vector.tensor_add(
                tmp[:, :, :I1 // 2], u_hat[:, :, :I1 // 2],
                u_hat[:, :, I1 // 2:],
            )
        else:
            # |b| stays < ~20 for this distribution: exp is fp32-safe
            # without the max-subtraction
            cexp = small.tile([128, I1, N_CAPS], f32)
            nc.scalar.activation(
                out=cexp[:], in_=b_log[:],
                func=mybir.ActivationFunctionType.Exp,
            )
            csum = small.tile([128, I1], f32)
            nc.vector.reduce_sum(
                out=csum[:], in_=cexp[:], axis=mybir.AxisListType.X
            )
            nc.vector.reciprocal(out=csum[:], in_=csum[:])
            c_t = small.tile([128, I1, N_CAPS], bf16)
            nc.vector.tensor_mul(
                c_t[:], cexp[:],
                csum.unsqueeze(2).broadcast_to([128, I1, N_CAPS]),
            )
            # s-mul: broadcast c over outermost d keeps bf16 2x mode
            nc.vector.tensor_mul(
                tmp[:], u_hat[:],
                c_t.unsqueeze(1).broadcast_to([128, OUT_DIM, I1, N_CAPS]),
            )
            nc.vector.tensor_add(
                tmp[:, :, :I1 // 2], tmp[:, :, :I1 // 2], tmp[:, :, I1 // 2:]
            )
        # contiguous in-place tree over i1 (middle dim)
        w = I1 // 2
        while w > 2:
            nc.vector.tensor_add(
                tmp[:, :, :w // 2], tmp[:, :, :w // 2], tmp[:, :, w // 2:w]
            )
            w //= 2
        # final tree level split per partition half: the upper half lands
        # first so its DMA to the bounce buffer overlaps the lower half's
        # add; the partition fold happens in the second DMA via the CCE
        # accumulate in the SDMA datapath
        s_half = small.tile([128, OUT_DIM, N_CAPS], bf16)
        nc.vector.tensor_add(
            s_half[B:2 * B], tmp[B:2 * B, :, 0, :], tmp[B:2 * B, :, 1, :]
        )
        cc_in = dram.tile([B, OUT_DIM, N_CAPS], bf16)
        cc_out = dram.tile(
            [B, OUT_DIM, N_CAPS], bf16, addr_space="Shared"
        )
        nc.sync.dma_start(cc_in[:], s_half[B:2 * B])
        nc.vector.tensor_add(
            s_half[0:B], tmp[0:B, :, 0, :], tmp[0:B, :, 1, :]
        )
        nc.gpsimd.dma_start(
            cc_in[:], s_half[0:B], accum_op=mybir.AluOpType.add
        )
        nc.gpsimd.collective_compute(
            "AllReduce",
            mybir.AluOpType.add,
            replica_groups=[list(range(N_CORES))],
            ins=[cc_in.opt()],
            outs=[cc_out.opt()],
        )
        # readback: both partition halves get the full sum so squash and the
        # next b-update run on all 128 partitions with no further shifts
        # (final iteration only needs the lower half)
        last = r == R - 1
        PH = B if last else 2 * B
        # readback on sync+gpsimd: the scalar queue may be busy with the
        # ACT_TABLE_LOAD for the upcoming sqrt and would delay its DMA
        s_sb = gath.tile([128, OUT_DIM, N_CAPS], bf16)
        nc.sync.dma_start(s_sb[0:B], cc_out[:])
        if not last:
            nc.gpsimd.dma_start(s_sb[B:2 * B], cc_out[:])
        # iteration 0's uniform c = 1/16 is folded into the squash math:
        # with s' = 16*s, ss = sum_d (s'/16)^2 and out = f(ss) * (s'/16)
        inv = 1.0 / N_CAPS if r == 0 else 1.0

        # squash: scale = ss/(1+ss)/sqrt(ss+eps), ss = sum_d s^2
        # (sq in bf16 keeps the 2-src op in the DVE 2x mode; the ss
        # accumulation below stays f32)
        sq = small.tile([128, OUT_DIM, N_CAPS], bf16)
        nc.vector.scalar_tensor_tensor(
            sq[0:PH], s_sb[0:PH], inv * inv, s_sb[0:PH],
            mybir.AluOpType.mult, mybir.AluOpType.mult,
        )
        ss = small.tile([128, N_CAPS], f32)
        nc.vector.reduce_sum(
            out=ss[0:PH], in_=sq[0:PH].transpose([0, 2, 1]),
            axis=mybir.AxisListType.X,
        )
        t1 = small.tile([128, N_CAPS], f32)
        nc.scalar.activation(
            out=t1[0:PH], in_=ss[0:PH],
            func=mybir.ActivationFunctionType.Sqrt,
            bias=eps_t[0:PH], scale=1.0,
        )
        t2 = small.tile([128, N_CAPS], f32)
        nc.vector.scalar_tensor_tensor(
            t2[0:PH], ss[0:PH], 1.0, t1[0:PH],
            mybir.AluOpType.add, mybir.AluOpType.mult,
        )   # (1+ss)*sqrt(ss+eps)
        nc.vector.reciprocal(out=t2[0:PH], in_=t2[0:PH])
        nc.vector.scalar_tensor_tensor(
            t1[0:PH], ss[0:PH], inv, t2[0:PH],
            mybir.AluOpType.mult, mybir.AluOpType.mult,
        )   # t1 = (ss*inv) / ((1+ss)*sqrt(ss+eps)) -- squash scale (*inv)
        if r == R - 1:
            out_t = small.tile([B, N_CAPS, OUT_DIM], f32)
            nc.vector.tensor_mul(
                out_t[:], s_sb[0:B].transpose([0, 2, 1]),
                t1[0:B].unsqueeze(2).broadcast_to([B, N_CAPS, OUT_DIM]),
            )
            nc.sync.dma_start(out[:], out_t[:])
        else:
            nc.vector.tensor_mul(
                out2[:], s_sb[:],
                t1.unsqueeze(1).broadcast_to([128, OUT_DIM, N_CAPS]),
            )
            # bu-mul: broadcast outputs over middle i1 keeps bf16 2x mode
            nc.vector.tensor_mul(
                tmp[:], u_hat[:],
                out2.unsqueeze(2).broadcast_to([128, OUT_DIM, I1, N_CAPS]),
            )
            w = OUT_DIM
            while w > 2:
                nc.vector.tensor_add(
                    tmp[:, :w // 2], tmp[:, :w // 2], tmp[:, w // 2:w]
                )
                w //= 2
            if r == 0:
                # b was zero: write the fresh logits directly
                nc.vector.tensor_add(b_log[:], tmp[:, 0], tmp[:, 1])
            else:
                bred = small.tile([128, I1, N_CAPS], f32)
                nc.vector.tensor_add(bred[:], tmp[:, 0], tmp[:, 1])
                nc.vector.tensor_add(b_log[:], b_log[:], bred[:])

    ctx.close()


def _build(num_routing):
    import concourse.bacc as bacc
    import concourse.tile as tile
    from concourse import mybir

    nc = bacc.Bacc(
        "TRN2", target_bir_lowering=False, debug=False, num_devices=N_CORES,
        dynamic_dma_scratch_size=512,
    )
    f32 = mybir.dt.float32
    bf16 = mybir.dt.bfloat16
    xT = nc.dram_tensor("xT", [IN_DIM, I_LOC, B], bf16, kind="ExternalInput")
    wT = nc.dram_tensor(
        "wT", [NGRP, IN_DIM, GRP, OUT_DIM, N_CAPS], bf16, kind="ExternalInput"
    )
    out = nc.dram_tensor(
        "out", [B, N_CAPS, OUT_DIM], f32, kind="ExternalOutput"
    )
    with tile.TileContext(nc) as tc:
        _emit(tc, xT, wT, out, num_routing)
    nc.compile()
    return nc


def kernel(inputs, W, num_routing):
    import ml_dtypes

    from concourse.bass_utils import run_bass_kernel_spmd

    R = int(num_routing)
    assert R >= 1
    if R not in _cache:
        _cache[R] = _build(R)
    nc = _cache[R]

    bf = ml_dtypes.bfloat16
    inputs = np.ascontiguousarray(np.asarray(inputs, dtype=np.float32))
    W = np.asarray(W, dtype=np.float32)

    in_maps = []
    for c in range(N_CORES):
        lo, hi = c * I_LOC, (c + 1) * I_LOC
        xT_c = np.ascontiguousarray(
            inputs[:, lo:hi, :].transpose(2, 1, 0).astype(bf)
        )
        # [i,j,k,d] -> group-blocked [g, k, t, d, j] so each group DMA is one
        # contiguous block and PSUM columns come out in (d, j) order
        wT_c = np.ascontiguousarray(
            W[lo:hi]
            .reshape(NGRP, GRP, N_CAPS, IN_DIM, OUT_DIM)
            .transpose(0, 3, 1, 4, 2)
            .astype(bf)
        )
        in_maps.append({"xT": xT_c, "wT": wT_c})

    kwargs = {}
    if TRACE:
        kwargs["trace"] = True
        if TRACE_DIR:
            kwargs["tmpdir"] = TRACE_DIR
    res = None
    for attempt in range(3):
        try:
            res = run_bass_kernel_spmd(
                nc, in_maps, core_ids=list(range(N_CORES)), **kwargs
            )
            break
        except Exception:
            if attempt == 2:
                raise
            import time
            time.sleep(5)
    if TRACE:
        kernel.last_exec_time_ns = res.exec_time_ns
        kernel.last_results = res
    return np.asarray(res.results[0]["out"], dtype=np.float32)


# revision 37
# speedup vs baseline: 1.0774x; 1.0613x over previous
"""CapsuleLayer (dynamic routing) Trainium2 kernel, SPMD over 8 NeuronCores.

Sharding: input-capsule axis (IN_CAPS=512 -> 64 per core). W and u_hat are
i-sharded; the bij,bijd->bjd contraction is completed with a bf16 AllReduce
of folded s-partials (64x16x32, 65 KB/rank) once per routing iteration. The
CCE in the SDMA datapath does the rank-sum, so readback is two contiguous
DMAs straight into both partition halves.

Per-core layout (i_local = i2*32 + i1, i2 in {0,1}):
  u_hat SBUF [p=(i2*64+b), (d, i1, j)] bf16 -- both broadcast multiplies
        (c over outermost d, outputs over middle i1) keep the DVE bf16 fast
        mode, and both contractions are contiguous in-place tree-adds
  b/c logits [p, (i1, j)]; s/outputs live on all 128 partitions (both halves
        hold the full batch) so squash and the b-update need no
        partition-shift inside the loop.

Phase 1 (per i): u_hat_i[b, dj] = xT_i.T @ W_i on the PE (K=128, M=64,
N=512), all in bf16.

The partition-half fold of the s-partial rides the bounce-buffer DMAs: the
upper half DMAs to DRAM while the lower half's add runs, then the lower
half lands with a CCE-accumulate DMA. A tiny warmup AllReduce issued at
kernel start absorbs the collective stack's ~60us background init plus the
first-collective premium off the critical path (measured: without it the
first real collective pays ~22us instead of ~9).
"""

import numpy as np

N_CORES = 8
B = 64
IN_CAPS = 512
IN_DIM = 128
N_CAPS = 16
OUT_DIM = 32
I_LOC = IN_CAPS // N_CORES          # 64 input capsules per core
I1 = 32                             # i_local = i2*32 + i1
JD = N_CAPS * OUT_DIM               # 512
EPS = 1e-7
GRP = 4                             # i's per W-DMA/PSUM group
NGRP = I_LOC // GRP                 # 16

# Toggled by test.py for profiling runs.
TRACE = False
TRACE_DIR = None

_cache = {}


def _emit(tc, xT, wT, out, num_routing):
    from contextlib import ExitStack

    from concourse import mybir

    nc = tc.nc
    f32 = mybir.dt.float32
    bf16 = mybir.dt.bfloat16
    ctx = ExitStack()
    singles = ctx.enter_context(tc.tile_pool(name="singles", bufs=1))
    wpool = ctx.enter_context(tc.tile_pool(name="wpool", bufs=4))
    pspool = ctx.enter_context(tc.tile_pool(name="pspool", bufs=2, space="PSUM"))
    small = ctx.enter_context(tc.tile_pool(name="small", bufs=2))
    gath = ctx.enter_context(tc.tile_pool(name="gath", bufs=1))
    dram = ctx.enter_context(tc.tile_pool(name="dram", bufs=2, space="DRAM"))

    # One tiny warmup collective: the collective stack finishes its background
    # init ~68us into the kernel and charges a first-collective premium; a
    # tiny collective absorbs both off the critical path (measured: without
    # it the r0 collective pays ~22us instead of ~7). AllGather warms the
    # same ncfw/mesh path but completes ~3us faster than an AllReduce.
    warm_in = dram.tile([1, 32], f32)
    warm_out = dram.tile([8, 32], f32)
    nc.gpsimd.collective_compute(
        "AllGather",
        mybir.AluOpType.bypass,
        replica_groups=[list(range(N_CORES))],
        ins=[warm_in.opt()],
        outs=[warm_out.opt()],
    )

    # ---- phase 1: u_hat = einsum over k, per local capsule i ----
    xsb = singles.tile([IN_DIM, I_LOC, B], bf16)         # [k, i, b]
    u_hat = singles.tile([128, OUT_DIM, I1, N_CAPS], bf16)  # [(i2,b), d, i1, j]

    XCH = I_LOC // 4
    for g in range(NGRP):
        i2 = (g * GRP) // I1
        i1g = (g * GRP) % I1
        # interleave the x chunks with the first W groups so the first
        # matmul's operands land as early as possible
        if g < 4:
            q = g
            nc.sync.dma_start(
                xsb[:, q * XCH:(q + 1) * XCH, :],
                xT[:, q * XCH:(q + 1) * XCH, :],
            )
        wtile = wpool.tile([IN_DIM, GRP, OUT_DIM, N_CAPS], bf16)
        nc.sync.dma_start(wtile[:], wT[g])
        ps = pspool.tile([128, GRP, OUT_DIM, N_CAPS], f32)
        for t in range(GRP):
            i = g * GRP + t
            nc.tensor.matmul(
                ps[i2 * B:(i2 + 1) * B, t], xsb[:, i, :], wtile[:, t],
                start=True, stop=True,
            )
        # copy+cast PSUM f32 -> SBUF bf16; dst viewed (i1, d, j) to match src
        dst = u_hat[i2 * B:(i2 + 1) * B, :, i1g:i1g + GRP, :].transpose(
            [0, 2, 1, 3]
        )
        src = ps[i2 * B:(i2 + 1) * B]
        if g % 2 == 0:
            nc.vector.tensor_copy(out=dst, in_=src)
        else:
            nc.scalar.copy(out=dst, in_=src)

    # ---- phase 2: routing ----
    tmp = singles.tile([128, OUT_DIM, I1, N_CAPS], bf16)
    b_log = singles.tile([128, I1, N_CAPS], f32)
    out2 = singles.tile([128, OUT_DIM, N_CAPS], bf16)    # squash(s) as (d, j)
    eps_t = singles.tile([128, 1], f32)
    nc.vector.memset(b_log[:], 0.0)
    nc.vector.memset(eps_t[:], EPS)

    R = num_routing
    for r in range(R):
        if r == 0:
            # b == 0 -> c uniform: s = (1/16) * sum_i u_hat (scale after AG)
            nc.vector.tensor_add(
                tmp[:, :, :I1 // 2], u_hat[:, :, :I1 // 2],
                u_hat[:, :, I1 // 2:],
            )
        else:
            # |b| stays < ~20 for this distribution: exp is fp32-safe
            # without the max-subtraction
            cexp = small.tile([128, I1, N_CAPS], f32)
            nc.scalar.activation(
                out=cexp[:], in_=b_log[:],
                func=mybir.ActivationFunctionType.Exp,
            )
            csum = small.tile([128, I1], f32)
            nc.vector.reduce_sum(
                out=csum[:], in_=cexp[:], axis=mybir.AxisListType.X
            )
            nc.vector.reciprocal(out=csum[:], in_=csum[:])
            c_t = small.tile([128, I1, N_CAPS], bf16)
            nc.vector.tensor_mul(
                c_t[:], cexp[:],
                csum.unsqueeze(2).broadcast_to([128, I1, N_CAPS]),
            )
            # s-mul: broadcast c over outermost d keeps bf16 2x mode
            nc.vector.tensor_mul(
                tmp[:], u_hat[:],
                c_t.unsqueeze(1).broadcast_to([128, OUT_DIM, I1, N_CAPS]),
            )
            nc.vector.tensor_add(
                tmp[:, :, :I1 // 2], tmp[:, :, :I1 // 2], tmp[:, :, I1 // 2:]
            )
        # contiguous in-place tree over i1 (middle dim)
        w = I1 // 2
        while w > 2:
            nc.vector.tensor_add(
                tmp[:, :, :w // 2], tmp[:, :, :w // 2], tmp[:, :, w // 2:w]
            )
            w //= 2
        # final tree level split per partition half: the upper half lands
        # first so its DMA to the bounce buffer overlaps the lower half's
        # add; the partition fold happens in the second DMA via the CCE
        # accumulate in the SDMA datapath
        s_half = small.tile([128, OUT_DIM, N_CAPS], bf16)
        nc.vector.tensor_add(
            s_half[B:2 * B], tmp[B:2 * B, :, 0, :], tmp[B:2 * B, :, 1, :]
        )
        cc_in = dram.tile([B, OUT_DIM, N_CAPS], bf16)
        cc_out = dram.tile(
            [B, OUT_DIM, N_CAPS], bf16, addr_space="Shared"
        )
        nc.sync.dma_start(cc_in[:], s_half[B:2 * B])
        nc.vector.tensor_add(
            s_half[0:B], tmp[0:B, :, 0, :], tmp[0:B, :, 1, :]
        )
        nc.gpsimd.dma_start(
            cc_in[:], s_half[0:B], accum_op=mybir.AluOpType.add
        )
        nc.gpsimd.collective_compute(
            "AllReduce",
            mybir.AluOpType.add,
            replica_groups=[list(range(N_CORES))],
            ins=[cc_in.opt()],
            outs=[cc_out.opt()],
        )
        # readback: both partition halves get the full sum so squash and the
        # next b-update run on all 128 partitions with no further shifts
        # (final iteration only needs the lower half)
        last = r == R - 1
        PH = B if last else 2 * B
        # readback on sync+gpsimd: the scalar queue may be busy with the
        # ACT_TABLE_LOAD for the upcoming sqrt and would delay its DMA
        s_sb = gath.tile([128, OUT_DIM, N_CAPS], bf16)
        nc.sync.dma_start(s_sb[0:B], cc_out[:])
        if not last:
            nc.gpsimd.dma_start(s_sb[B:2 * B], cc_out[:])
        # iteration 0's uniform c = 1/16 is folded into the squash math:
        # with s' = 16*s, ss = sum_d (s'/16)^2 and out = f(ss) * (s'/16)
        inv = 1.0 / N_CAPS if r == 0 else 1.0

        # squash: scale = ss/(1+ss)/sqrt(ss+eps), ss = sum_d s^2
        sq = small.tile([128, OUT_DIM, N_CAPS], f32)
        nc.vector.scalar_tensor_tensor(
            sq[0:PH], s_sb[0:PH], inv * inv, s_sb[0:PH],
            mybir.AluOpType.mult, mybir.AluOpType.mult,
        )
        ss = small.tile([128, N_CAPS], f32)
        nc.vector.reduce_sum(
            out=ss[0:PH], in_=sq[0:PH].transpose([0, 2, 1]),
            axis=mybir.AxisListType.X,
        )
        t1 = small.tile([128, N_CAPS], f32)
        nc.scalar.activation(
            out=t1[0:PH], in_=ss[0:PH],
            func=mybir.ActivationFunctionType.Sqrt,
            bias=eps_t[0:PH], scale=1.0,
        )
        t2 = small.tile([128, N_CAPS], f32)
        nc.vector.scalar_tensor_tensor(
            t2[0:PH], ss[0:PH], 1.0, t1[0:PH],
            mybir.AluOpType.add, mybir.AluOpType.mult,
        )   # (1+ss)*sqrt(ss+eps)
        nc.vector.reciprocal(out=t2[0:PH], in_=t2[0:PH])
        nc.vector.scalar_tensor_tensor(
            t1[0:PH], ss[0:PH], inv, t2[0:PH],
            mybir.AluOpType.mult, mybir.AluOpType.mult,
        )   # t1 = (ss*inv) / ((1+ss)*sqrt(ss+eps)) -- squash scale (*inv)
        if r == R - 1:
            out_t = small.tile([B, N_CAPS, OUT_DIM], f32)
            nc.vector.tensor_mul(
                out_t[:], s_sb[0:B].transpose([0, 2, 1]),
                t1[0:B].unsqueeze(2).broadcast_to([B, N_CAPS, OUT_DIM]),
            )
            nc.sync.dma_start(out[:], out_t[:])
        else:
            nc.vector.tensor_mul(
                out2[:], s_sb[:],
                t1.unsqueeze(1).broadcast_to([128, OUT_DIM, N_CAPS]),
            )
            # bu-mul: broadcast outputs over middle i1 keeps bf16 2x mode
            nc.vector.tensor_mul(
                tmp[:], u_hat[:],
                out2.unsqueeze(2).broadcast_to([128, OUT_DIM, I1, N_CAPS]),
            )
            w = OUT_DIM
            while w > 2:
                nc.vector.tensor_add(
                    tmp[:, :w // 2], tmp[:, :w // 2], tmp[:, w // 2:w]
                )
                w //= 2
            if r == 0:
                # b was zero: write the fresh logits directly
                nc.vector.tensor_add(b_log[:], tmp[:, 0], tmp[:, 1])
            else:
                bred = small.tile([128, I1, N_CAPS], f32)
                nc.vector.tensor_add(bred[:], tmp[:, 0], tmp[:, 1])
                nc.vector.tensor_add(b_log[:], b_log[:], bred[:])

    ctx.close()


def _build(num_routing):
    import concourse.bacc as bacc
    import concourse.tile as tile
    from concourse import mybir

    nc = bacc.Bacc(
        "TRN2", target_bir_lowering=False, debug=False, num_devices=N_CORES,
        dynamic_dma_scratch_size=512,
    )
    f32 = mybir.dt.float32
    bf16 = mybir.dt.bfloat16
    xT = nc.dram_tensor("xT", [IN_DIM, I_LOC, B], bf16, kind="ExternalInput")
    wT = nc.dram_tensor(
        "wT", [NGRP, IN_DIM, GRP, OUT_DIM, N_CAPS], bf16, kind="ExternalInput"
    )
    out = nc.dram_tensor(
        "out", [B, N_CAPS, OUT_DIM], f32, kind="ExternalOutput"
    )
    with tile.TileContext(nc) as tc:
        _emit(tc, xT, wT, out, num_routing)
    nc.compile()
    return nc


def kernel(inputs, W, num_routing):
    import ml_dtypes

    from concourse.bass_utils import run_bass_kernel_spmd

    R = int(num_routing)
    assert R >= 1
    if R not in _cache:
        _cache[R] = _build(R)
    nc = _cache[R]

    bf = ml_dtypes.bfloat16
    inputs = np.ascontiguousarray(np.asarray(inputs, dtype=np.float32))
    W = np.asarray(W, dtype=np.float32)

    in_maps = []
    for c in range(N_CORES):
        lo, hi = c * I_LOC, (c + 1) * I_LOC
        xT_c = np.ascontiguousarray(
            inputs[:, lo:hi, :].transpose(2, 1, 0).astype(bf)
        )
        # [i,j,k,d] -> group-blocked [g, k, t, d, j] so each group DMA is one
        # contiguous block and PSUM columns come out in (d, j) order
        wT_c = np.ascontiguousarray(
            W[lo:hi]
            .reshape(NGRP, GRP, N_CAPS, IN_DIM, OUT_DIM)
            .transpose(0, 3, 1, 4, 2)
            .astype(bf)
        )
        in_maps.append({"xT": xT_c, "wT": wT_c})

    kwargs = {}
    if TRACE:
        kwargs["trace"] = True
        if TRACE_DIR:
            kwargs["tmpdir"] = TRACE_DIR
    res = None
    for attempt in range(3):
        try:
            res = run_bass_kernel_spmd(
                nc, in_maps, core_ids=list(range(N_CORES)), **kwargs
            )
            break
        except Exception:
            if attempt == 2:
                raise
            import time
            time.sleep(5)
    if TRACE:
        kernel.last_exec_time_ns = res.exec_time_ns
        kernel.last_results = res
    return np.asarray(res.results[0]["out"], dtype=np.float32)
